# revision 2
# baseline (speedup 1.0000x reference)
"""Trainium2 Bass kernel for batched dynamic embedding table lookup.

Problem: indices [2, 4096, 20] int64, table0/table1 [1_000_000, 128] f32.
out[b, f*128:(f+1)*128] = sum_l table_f[indices[f, b, l], :]  -> [4096, 256] f32

Sharding (model/feature + batch parallel, no collectives):
  cores 0-3: table0, batch blocks of 1024 rows each
  cores 4-7: table1, batch blocks of 1024 rows each
Each core gathers 1024*20 = 20480 rows of 512B via indirect DMA and
sum-pools on the vector engine. Host only reshapes indices and
concatenates per-core outputs (pure layout, no arithmetic).
"""

import numpy as np

import concourse.bacc as bacc
import concourse.bass as bass
import concourse.tile as tile
from concourse import mybir

V = 1_000_000   # rows per table
D = 128         # embedding dim
L = 20          # multi-hot bag size
P = 128         # SBUF partitions
BC = 1024       # batch rows per core
A = BC // P     # batch rows per partition (8)
N_CORES = 8


def build_nc(v=V, d=D, l=L, a=A, gather_bufs=3):
    """Build the per-core Bass program (same program for all cores)."""
    nc = bacc.Bacc("TRN2", target_bir_lowering=False, debug=False)
    table = nc.dram_tensor("table", [v, d], mybir.dt.float32, kind="ExternalInput")
    idx = nc.dram_tensor("idx", [P, a * l], mybir.dt.int32, kind="ExternalInput")
    out = nc.dram_tensor("out", [P * a, d], mybir.dt.float32, kind="ExternalOutput")

    with tile.TileContext(nc) as tc:
        with (
            tc.tile_pool(name="io", bufs=1) as io_pool,
            tc.tile_pool(name="gather", bufs=gather_bufs) as gpool,
        ):
            idx_t = io_pool.tile([P, a * l], mybir.dt.int32)
            nc.sync.dma_start(idx_t[:], idx[:, :])
            pooled = io_pool.tile([P, a * d], mybir.dt.float32)
            for ai in range(a):
                g = gpool.tile([P, l * d], mybir.dt.float32)
                # HW indirect1d semantics: ONE index per partition per
                # instruction, each reading one contiguous d-row.
                for li in range(l):
                    nc.gpsimd.indirect_dma_start(
                        out=g[:, li * d : (li + 1) * d],
                        out_offset=None,
                        in_=table[:, :],
                        in_offset=bass.IndirectOffsetOnAxis(
                            ap=idx_t[:, ai * l + li : ai * l + li + 1],
                            axis=0,
                        ),
                    )
                # g[p, l*d + d'] = table[idx[p, ai*l + l'], d'] ; reduce over l
                gv = g[:].rearrange("p (l d) -> p d l", l=l)
                nc.vector.tensor_reduce(
                    out=pooled[:, ai * d : (ai + 1) * d],
                    in_=gv,
                    axis=mybir.AxisListType.X,
                    op=mybir.AluOpType.add,
                )
            # out row b = p*a + ai  ->  contiguous [P, a*d] view
            nc.sync.dma_start(
                out.rearrange("(p a) d -> p (a d)", p=P), pooled[:]
            )
    nc.compile()
    return nc


_NC_CACHE = None
LAST_RESULT = None  # test harness introspection


def _get_nc():
    global _NC_CACHE
    if _NC_CACHE is None:
        _NC_CACHE = build_nc()
    return _NC_CACHE


def kernel(indices, table0, table1):
    from concourse.bass_utils import run_bass_kernel_spmd

    global LAST_RESULT
    nc = _get_nc()
    indices = np.asarray(indices)
    t0 = np.ascontiguousarray(np.asarray(table0, dtype=np.float32))
    t1 = np.ascontiguousarray(np.asarray(table1, dtype=np.float32))

    in_maps = []
    for core in range(N_CORES):
        f = 0 if core < 4 else 1
        blk = core % 4
        sub = indices[f, blk * BC : (blk + 1) * BC, :].astype(np.int32)  # [BC, L]
        # partition p holds batches p*A .. p*A+A-1; slot i = (b%A)*L + l
        idx_host = np.ascontiguousarray(sub.reshape(P, A * L))
        in_maps.append({"table": t0 if f == 0 else t1, "idx": idx_host})

    LAST_RESULT = run_bass_kernel_spmd(nc, in_maps, core_ids=list(range(N_CORES)))
    outs = [r["out"] for r in LAST_RESULT.results]
    pooled0 = np.concatenate(outs[0:4], axis=0)  # [4096, 128]
    pooled1 = np.concatenate(outs[4:8], axis=0)  # [4096, 128]
    return np.concatenate([pooled0, pooled1], axis=1).astype(np.float32)


# revision 4
# speedup vs baseline: 1.0053x; 1.0053x over previous
"""Trainium2 Bass kernel for batched dynamic embedding table lookup.

Problem: indices [2, 4096, 20] int64, table0/table1 [1_000_000, 128] f32.
out[b, f*128:(f+1)*128] = sum_l table_f[indices[f, b, l], :]  -> [4096, 256] f32

Sharding (model/feature + batch parallel, no collectives):
  cores 0-3: table0, batch blocks of 1024 rows each
  cores 4-7: table1, batch blocks of 1024 rows each
Each core gathers 1024*20 = 20480 rows of 512B via indirect DMA and
sum-pools on the vector engine. Host only reshapes indices and
concatenates per-core outputs (pure layout, no arithmetic).
"""

import numpy as np

import concourse.bacc as bacc
import concourse.bass as bass
import concourse.tile as tile
from concourse import mybir

V = 1_000_000   # rows per table
D = 128         # embedding dim
L = 20          # multi-hot bag size
P = 128         # SBUF partitions
BC = 1024       # batch rows per core
A = BC // P     # batch rows per partition (8)
N_CORES = 8


def build_nc(v=V, d=D, l=L, a=A, scratch=65536):
    """Build the per-core Bass program (same program for all cores)."""
    nc = bacc.Bacc(
        "TRN2",
        target_bir_lowering=False,
        debug=False,
        dynamic_dma_scratch_size=scratch,
    )
    table = nc.dram_tensor("table", [v, d], mybir.dt.float32, kind="ExternalInput")
    idx = nc.dram_tensor("idx", [P, a * l], mybir.dt.int32, kind="ExternalInput")
    out = nc.dram_tensor("out", [P * a, d], mybir.dt.float32, kind="ExternalOutput")

    with tile.TileContext(nc) as tc:
        with (
            tc.tile_pool(name="io", bufs=1) as io_pool,
            tc.tile_pool(name="gather", bufs=1) as gpool,
        ):
            idx_t = io_pool.tile([P, a * l], mybir.dt.int32)
            nc.sync.dma_start(idx_t[:], idx[:, :])
            pooled = io_pool.tile([P, a * d], mybir.dt.float32)
            # One giant gather tile: no slot recycling, so the 160 indirect
            # DMAs stream back-to-back on the Pool engine with no tile-reuse
            # waits; per-a reduces overlap trailing gathers on DVE.
            g = gpool.tile([P, a * l * d], mybir.dt.float32)
            for ai in range(a):
                # HW indirect1d semantics: ONE index per partition per
                # instruction, each reading one contiguous d-row.
                for li in range(l):
                    s = ai * l + li
                    nc.gpsimd.indirect_dma_start(
                        out=g[:, s * d : (s + 1) * d],
                        out_offset=None,
                        in_=table[:, :],
                        in_offset=bass.IndirectOffsetOnAxis(
                            ap=idx_t[:, s : s + 1],
                            axis=0,
                        ),
                    )
                # g[p, (a l) d] = table[idx[p, a*20+l], :] ; reduce over l
                gv = g[:, ai * l * d : (ai + 1) * l * d].rearrange(
                    "p (l d) -> p d l", l=l
                )
                nc.vector.tensor_reduce(
                    out=pooled[:, ai * d : (ai + 1) * d],
                    in_=gv,
                    axis=mybir.AxisListType.X,
                    op=mybir.AluOpType.add,
                )
            # out row b = p*a + ai  ->  contiguous [P, a*d] view
            nc.sync.dma_start(
                out.rearrange("(p a) d -> p (a d)", p=P), pooled[:]
            )
    nc.compile()
    return nc


_NC_CACHE = None
LAST_RESULT = None  # test harness introspection


def _get_nc():
    global _NC_CACHE
    if _NC_CACHE is None:
        _NC_CACHE = build_nc()
    return _NC_CACHE


def _ensure_axon_hooks():
    """bass_utils' axon trace path imports antenv.axon_hooks, which this
    image's antenv package lacks. Install a stub so a BASS_TRACE=1 env
    doesn't crash the run (hook=None -> tracing skipped gracefully)."""
    import sys
    import types

    if "antenv.axon_hooks" in sys.modules:
        return
    try:
        import antenv
    except ImportError:
        return
    if hasattr(antenv, "axon_hooks"):
        sys.modules.setdefault("antenv.axon_hooks", antenv.axon_hooks)
        return
    mod = types.ModuleType("antenv.axon_hooks")
    holder = [None]
    mod.set_axon_ntff_profile_hook = lambda h: holder.__setitem__(0, h)
    mod.get_axon_ntff_profile_hook = lambda: holder[0]
    sys.modules["antenv.axon_hooks"] = mod
    antenv.axon_hooks = mod


def kernel(indices, table0, table1):
    from concourse.bass_utils import run_bass_kernel_spmd

    _ensure_axon_hooks()

    global LAST_RESULT
    nc = _get_nc()
    indices = np.asarray(indices)
    t0 = np.ascontiguousarray(np.asarray(table0, dtype=np.float32))
    t1 = np.ascontiguousarray(np.asarray(table1, dtype=np.float32))

    in_maps = []
    for core in range(N_CORES):
        f = 0 if core < 4 else 1
        blk = core % 4
        sub = indices[f, blk * BC : (blk + 1) * BC, :].astype(np.int32)  # [BC, L]
        # partition p holds batches p*A .. p*A+A-1; slot i = (b%A)*L + l
        idx_host = np.ascontiguousarray(sub.reshape(P, A * L))
        in_maps.append({"table": t0 if f == 0 else t1, "idx": idx_host})

    LAST_RESULT = run_bass_kernel_spmd(nc, in_maps, core_ids=list(range(N_CORES)))
    outs = [r["out"] for r in LAST_RESULT.results]
    pooled0 = np.concatenate(outs[0:4], axis=0)  # [4096, 128]
    pooled1 = np.concatenate(outs[4:8], axis=0)  # [4096, 128]
    return np.concatenate([pooled0, pooled1], axis=1).astype(np.float32)


# revision 5
# speedup vs baseline: 1.0096x; 1.0043x over previous
"""Trainium2 Bass kernel for batched dynamic embedding table lookup.

Problem: indices [2, 4096, 20] int64, table0/table1 [1_000_000, 128] f32.
out[b, f*128:(f+1)*128] = sum_l table_f[indices[f, b, l], :]  -> [4096, 256] f32

Sharding (model/feature + batch parallel, no collectives):
  cores 0-3: table0, batch blocks of 1024 rows each
  cores 4-7: table1, batch blocks of 1024 rows each
Each core gathers 1024*20 = 20480 rows of 512B via indirect DMA and
sum-pools on the vector engine. Host only reshapes indices and
concatenates per-core outputs (pure layout, no arithmetic).
"""

import numpy as np

import concourse.bacc as bacc
import concourse.bass as bass
import concourse.tile as tile
from concourse import mybir

V = 1_000_000   # rows per table
D = 128         # embedding dim
L = 20          # multi-hot bag size
P = 128         # SBUF partitions
BC = 1024       # batch rows per core
A = BC // P     # batch rows per partition (8)
N_CORES = 8


def build_nc(v=V, d=D, l=L, a=A, scratch=65536):
    """Build the per-core Bass program (same program for all cores)."""
    nc = bacc.Bacc(
        "TRN2",
        target_bir_lowering=False,
        debug=False,
        dynamic_dma_scratch_size=scratch,
    )
    table = nc.dram_tensor("table", [v, d], mybir.dt.float32, kind="ExternalInput")
    idx = nc.dram_tensor("idx", [P, a * l], mybir.dt.int32, kind="ExternalInput")
    out = nc.dram_tensor("out", [P * a, d], mybir.dt.float32, kind="ExternalOutput")

    with tile.TileContext(nc) as tc:
        with (
            tc.tile_pool(name="io", bufs=1) as io_pool,
            tc.tile_pool(name="gather", bufs=1) as gpool,
        ):
            idx_t = io_pool.tile([P, a * l], mybir.dt.int32)
            nc.sync.dma_start(idx_t[:], idx[:, :])
            pooled = io_pool.tile([P, a * d], mybir.dt.float32)
            # One giant gather tile: no slot recycling, so the 160 indirect
            # DMAs stream back-to-back on the Pool engine with no tile-reuse
            # waits; per-a reduces overlap trailing gathers on DVE.
            g = gpool.tile([P, a * l * d], mybir.dt.float32)

            def gather(s):
                nc.gpsimd.indirect_dma_start(
                    out=g[:, s * d : (s + 1) * d],
                    out_offset=None,
                    in_=table[:, :],
                    in_offset=bass.IndirectOffsetOnAxis(
                        ap=idx_t[:, s : s + 1],
                        axis=0,
                    ),
                )

            def reduce_slots(out_ap, s0, nl):
                gv = g[:, s0 * d : (s0 + nl) * d].rearrange(
                    "p (l d) -> p d l", l=nl
                )
                nc.vector.tensor_reduce(
                    out=out_ap,
                    in_=gv,
                    axis=mybir.AxisListType.X,
                    op=mybir.AluOpType.add,
                )

            # HW indirect1d semantics: ONE index per partition per
            # instruction, each reading one contiguous d-row.
            for ai in range(a - 1):
                for li in range(l):
                    gather(ai * l + li)
                # g[p, (a l) d] = table[idx[p, a*20+l], :] ; reduce over l
                reduce_slots(pooled[:, ai * d : (ai + 1) * d], ai * l, l)
            out_v = out.rearrange("(p a) d -> p (a d)", p=P)
            # groups 0..a-2 store early, fully hidden under the gathers
            nc.sync.dma_start(out_v[:, : (a - 1) * d], pooled[:, : (a - 1) * d])

            # Last group: progressive partial reduces so only a 5-slot
            # partial + one add trail the final gather DMA.
            ai = a - 1
            half = l // 2
            q = l // 4
            part = io_pool.tile([P, 2 * d], mybir.dt.float32)
            for li in range(half):
                gather(ai * l + li)
            reduce_slots(part[:, :d], ai * l, half)  # l 0..9
            for li in range(half, l - q):
                gather(ai * l + li)
            reduce_slots(part[:, d:], ai * l + half, half - q)  # l 10..14
            nc.vector.tensor_add(
                out=part[:, :d], in0=part[:, :d], in1=part[:, d:]
            )
            for li in range(l - q, l):
                gather(ai * l + li)
            reduce_slots(part[:, d:], ai * l + l - q, q)  # l 15..19
            nc.vector.tensor_add(
                out=pooled[:, ai * d : (ai + 1) * d],
                in0=part[:, :d],
                in1=part[:, d:],
            )
            nc.sync.dma_start(
                out_v[:, ai * d :], pooled[:, ai * d : (ai + 1) * d]
            )
    nc.compile()
    return nc


_NC_CACHE = None
LAST_RESULT = None  # test harness introspection


def _get_nc():
    global _NC_CACHE
    if _NC_CACHE is None:
        _NC_CACHE = build_nc()
    return _NC_CACHE


def _ensure_axon_hooks():
    """bass_utils' axon trace path imports antenv.axon_hooks, which this
    image's antenv package lacks. Install a stub so a BASS_TRACE=1 env
    doesn't crash the run (hook=None -> tracing skipped gracefully)."""
    import sys
    import types

    if "antenv.axon_hooks" in sys.modules:
        return
    try:
        import antenv
    except ImportError:
        return
    if hasattr(antenv, "axon_hooks"):
        sys.modules.setdefault("antenv.axon_hooks", antenv.axon_hooks)
        return
    mod = types.ModuleType("antenv.axon_hooks")
    holder = [None]
    mod.set_axon_ntff_profile_hook = lambda h: holder.__setitem__(0, h)
    mod.get_axon_ntff_profile_hook = lambda: holder[0]
    sys.modules["antenv.axon_hooks"] = mod
    antenv.axon_hooks = mod


def kernel(indices, table0, table1):
    from concourse.bass_utils import run_bass_kernel_spmd

    _ensure_axon_hooks()

    global LAST_RESULT
    nc = _get_nc()
    indices = np.asarray(indices)
    t0 = np.ascontiguousarray(np.asarray(table0, dtype=np.float32))
    t1 = np.ascontiguousarray(np.asarray(table1, dtype=np.float32))

    in_maps = []
    for core in range(N_CORES):
        f = 0 if core < 4 else 1
        blk = core % 4
        sub = indices[f, blk * BC : (blk + 1) * BC, :].astype(np.int32)  # [BC, L]
        # partition p holds batches p*A .. p*A+A-1; slot i = (b%A)*L + l
        idx_host = np.ascontiguousarray(sub.reshape(P, A * L))
        in_maps.append({"table": t0 if f == 0 else t1, "idx": idx_host})

    LAST_RESULT = run_bass_kernel_spmd(nc, in_maps, core_ids=list(range(N_CORES)))
    outs = [r["out"] for r in LAST_RESULT.results]
    pooled0 = np.concatenate(outs[0:4], axis=0)  # [4096, 128]
    pooled1 = np.concatenate(outs[4:8], axis=0)  # [4096, 128]
    return np.concatenate([pooled0, pooled1], axis=1).astype(np.float32)


# revision 6
# speedup vs baseline: 1.0267x; 1.0170x over previous
"""Trainium2 Bass kernel for batched dynamic embedding table lookup.

Problem: indices [2, 4096, 20] int64, table0/table1 [1_000_000, 128] f32.
out[b, f*128:(f+1)*128] = sum_l table_f[indices[f, b, l], :]  -> [4096, 256] f32

Sharding (model/feature + batch parallel, no collectives):
  cores 0-3: table0, batch blocks of 1024 rows each
  cores 4-7: table1, batch blocks of 1024 rows each
Each core gathers 1024*20 = 20480 rows of 512B via indirect DMA and
sum-pools on the vector engine. Host only reshapes indices and
concatenates per-core outputs (pure layout, no arithmetic).
"""

import numpy as np

import concourse.bacc as bacc
import concourse.bass as bass
import concourse.tile as tile
from concourse import mybir

V = 1_000_000   # rows per table
D = 128         # embedding dim
L = 20          # multi-hot bag size
P = 128         # SBUF partitions
BC = 1024       # batch rows per core
A = BC // P     # batch rows per partition (8)
N_CORES = 8


def build_nc(v=V, d=D, l=L, a=A, scratch=65536):
    """Build the per-core Bass program (same program for all cores)."""
    nc = bacc.Bacc(
        "TRN2",
        target_bir_lowering=False,
        debug=False,
        dynamic_dma_scratch_size=scratch,
    )
    table = nc.dram_tensor("table", [v, d], mybir.dt.float32, kind="ExternalInput")
    idx = nc.dram_tensor("idx", [P, a * l], mybir.dt.int32, kind="ExternalInput")
    out = nc.dram_tensor("out", [P * a, d], mybir.dt.float32, kind="ExternalOutput")

    with tile.TileContext(nc) as tc:
        with (
            tc.tile_pool(name="io", bufs=1) as io_pool,
            tc.tile_pool(name="gather", bufs=1) as gpool,
        ):
            idx_t = io_pool.tile([P, a * l], mybir.dt.int32)
            nc.sync.dma_start(idx_t[:], idx[:, :])
            pooled = io_pool.tile([P, a * d], mybir.dt.float32)
            # One giant gather tile: no slot recycling, so the 160 indirect
            # DMAs stream back-to-back on the Pool engine with no tile-reuse
            # waits; per-a reduces overlap trailing gathers on DVE.
            g = gpool.tile([P, a * l * d], mybir.dt.float32)

            def gather(s):
                nc.gpsimd.indirect_dma_start(
                    out=g[:, s * d : (s + 1) * d],
                    out_offset=None,
                    in_=table[:, :],
                    in_offset=bass.IndirectOffsetOnAxis(
                        ap=idx_t[:, s : s + 1],
                        axis=0,
                    ),
                )

            def reduce_slots(out_ap, s0, nl):
                gv = g[:, s0 * d : (s0 + nl) * d].rearrange(
                    "p (l d) -> p d l", l=nl
                )
                nc.vector.tensor_reduce(
                    out=out_ap,
                    in_=gv,
                    axis=mybir.AxisListType.X,
                    op=mybir.AluOpType.add,
                )

            # HW indirect1d semantics: ONE index per partition per
            # instruction, each reading one contiguous d-row.
            for ai in range(a - 1):
                for li in range(l):
                    gather(ai * l + li)
                # g[p, (a l) d] = table[idx[p, a*20+l], :] ; reduce over l
                reduce_slots(pooled[:, ai * d : (ai + 1) * d], ai * l, l)
            out_v = out.rearrange("(p a) d -> p (a d)", p=P)
            # groups 0..a-2 store early, fully hidden under the gathers
            nc.sync.dma_start(out_v[:, : (a - 1) * d], pooled[:, : (a - 1) * d])

            # Last group: progressive partial reduces so only a 2-slot
            # partial + one add trail the final gather DMA.
            ai = a - 1
            c1, c2 = l // 2, l - 2 - l // 2  # 10, 8 -> final partial = 2
            part = io_pool.tile([P, 2 * d], mybir.dt.float32)
            for li in range(c1):
                gather(ai * l + li)
            reduce_slots(part[:, :d], ai * l, c1)  # l 0..9
            for li in range(c1, c1 + c2):
                gather(ai * l + li)
            reduce_slots(part[:, d:], ai * l + c1, c2)  # l 10..17
            nc.vector.tensor_add(
                out=part[:, :d], in0=part[:, :d], in1=part[:, d:]
            )
            for li in range(c1 + c2, l):
                gather(ai * l + li)
            reduce_slots(part[:, d:], ai * l + c1 + c2, l - c1 - c2)  # l 18..19
            nc.vector.tensor_add(
                out=pooled[:, ai * d : (ai + 1) * d],
                in0=part[:, :d],
                in1=part[:, d:],
            )
            nc.sync.dma_start(
                out_v[:, ai * d :], pooled[:, ai * d : (ai + 1) * d]
            )
    nc.compile()
    return nc


_NC_CACHE = None
LAST_RESULT = None  # test harness introspection


def _get_nc():
    global _NC_CACHE
    if _NC_CACHE is None:
        _NC_CACHE = build_nc()
    return _NC_CACHE


def _ensure_axon_hooks():
    """bass_utils' axon trace path imports antenv.axon_hooks, which this
    image's antenv package lacks. Install a stub so a BASS_TRACE=1 env
    doesn't crash the run (hook=None -> tracing skipped gracefully)."""
    import sys
    import types

    if "antenv.axon_hooks" in sys.modules:
        return
    try:
        import antenv
    except ImportError:
        return
    if hasattr(antenv, "axon_hooks"):
        sys.modules.setdefault("antenv.axon_hooks", antenv.axon_hooks)
        return
    mod = types.ModuleType("antenv.axon_hooks")
    holder = [None]
    mod.set_axon_ntff_profile_hook = lambda h: holder.__setitem__(0, h)
    mod.get_axon_ntff_profile_hook = lambda: holder[0]
    sys.modules["antenv.axon_hooks"] = mod
    antenv.axon_hooks = mod


def kernel(indices, table0, table1):
    from concourse.bass_utils import run_bass_kernel_spmd

    _ensure_axon_hooks()

    global LAST_RESULT
    nc = _get_nc()
    indices = np.asarray(indices)
    t0 = np.ascontiguousarray(np.asarray(table0, dtype=np.float32))
    t1 = np.ascontiguousarray(np.asarray(table1, dtype=np.float32))

    in_maps = []
    for core in range(N_CORES):
        f = 0 if core < 4 else 1
        blk = core % 4
        sub = indices[f, blk * BC : (blk + 1) * BC, :].astype(np.int32)  # [BC, L]
        # partition p holds batches p*A .. p*A+A-1; slot i = (b%A)*L + l
        idx_host = np.ascontiguousarray(sub.reshape(P, A * L))
        in_maps.append({"table": t0 if f == 0 else t1, "idx": idx_host})

    LAST_RESULT = run_bass_kernel_spmd(nc, in_maps, core_ids=list(range(N_CORES)))
    outs = [r["out"] for r in LAST_RESULT.results]
    pooled0 = np.concatenate(outs[0:4], axis=0)  # [4096, 128]
    pooled1 = np.concatenate(outs[4:8], axis=0)  # [4096, 128]
    return np.concatenate([pooled0, pooled1], axis=1).astype(np.float32)
